# revision 58
# baseline (speedup 1.0000x reference)
"""Symmetric Chamfer distance (Euclidean norm) on 8 Trainium2 NeuronCores.

Problem: pc1, pc2: [B=4, N=4096, D=3] fp32. For each batch, the reference
materializes the [N, N] distance matrix dist[i, j] = ||a_i - b_j||_2, takes
row-mins and col-mins, and averages. Output: fp32 scalar.

Strategy
--------
Sharding: core c handles (batch b = c//2, half h = c%2) -> 2048 a-points
(rows of the distance matrix) x all 4096 b-points.

Math: d2(i,j) = |a_i|^2 + |b_j|^2 - 2 a_i.b_j, computed on the TensorEngine
as a K=13 fp16 matmul using a hi/lo fp16 split of every operand
(x = hi + lo, both fp16, so hi*hi + hi*lo + lo*hi captures the fp32 product
to ~2^-24): d2 comes out fp32-exact in PSUM at full fp16 matmul speed
(1 cycle/row vs 4 for fp32).

The u (per a-point) / v (per b-point) staging vectors of the K=13 product
are a tiny O(B*N*D) layout+precision transform of the inputs, done on host
as part of sharding.

Per [128, 4096] row-block of d2 (one i-tile):
  - PE: 8 matmuls of [13,128]x[13,512] -> PSUM fp32 (4-bank groups)
  - ScalarE: 2x activation-Copy with scale=-1.0 -> SBUF fp16 = NEGATED d2
    (negation turns every min into a max; host flips signs at the end)
  - VectorE: log2-fold max (fp16, 2x mode) -> per-a-point row maxes, plus a
    running elementwise max into acc[128, 4096] for the per-b-point column
    direction
  - tail: one blocked DMA-xbar transpose of acc, then DVE free-axis maxes
    finish the per-b-point column mins on device

VectorE is the bottleneck engine (~90% busy): every d2 value crosses it
twice (fold + acc) at 2 fp16 elem/lane/cycle, and no other engine on trn2
can do elementwise/reduction min through this toolchain (gpsimd software
tensor ops are rejected by walrus codegen for the Pool engine).

Min over fp16(d2) followed by host-side sqrt is exact enough: sqrt is
monotone so min commutes, and fp16 rounding of d2 gives ~5e-4 relative
per-element noise that averages out over 4096 mins (measured end-to-end
relative error ~5e-7 vs the fp32 reference).

Host combine: per batch, min the two half-shard column vectors, flip signs,
clamp, sqrt, sum - O(N) work.
"""

import numpy as np

_B, _N, _D = 4, 4096, 3
_NCORES = 8
_HALF = _N // 2  # a-points per core
_K = 13          # contraction slots of the split-fp16 quadratic expansion
_NT = _HALF // 128  # 16 i-tiles per core
_JC = 512        # j-chunk per matmul (one PSUM bank of fp32)

TRACE = False            # test harness may flip before calling kernel()
LAST_RESULT = None       # BassKernelResults of the last run (for profiling)

USE_DMAT_TAIL = True     # DMA-xbar transpose tail (else: PE transpose tail)

_prog_cache = None


def _build_program():
    import concourse.bass as bass
    import concourse.mybir as mybir
    from concourse import bacc, tile

    f16 = mybir.dt.float16
    f32 = mybir.dt.float32
    ts = bass.ts
    MAX = mybir.AluOpType.max

    nc = bacc.Bacc(
        "TRN2",
        target_bir_lowering=False,
        debug=False,
        num_devices=_NCORES,
    )
    u_d = nc.declare_dram_parameter("u", [_K, _HALF], f16, isOutput=False)
    v_d = nc.declare_dram_parameter("v", [_K, _N], f16, isOutput=False)
    eye_d = nc.declare_dram_parameter("eye", [128, 128], f16, isOutput=False)
    m1_d = nc.declare_dram_parameter("m1", [128, _NT], f16, isOutput=True)
    m2_d = nc.declare_dram_parameter("m2", [128, _N // 128], f16, isOutput=True)

    with tile.TileContext(nc) as tc:
        with (
            tc.tile_pool(name="const", bufs=1) as cpool,
            tc.tile_pool(name="dpool", bufs=6) as dpool,
            tc.tile_pool(name="fpool", bufs=3) as fpool,
            tc.tile_pool(name="gpool", bufs=3) as gpool,
            tc.tile_pool(name="psum", bufs=2, space="PSUM") as ppool,
        ):
            u_sb = cpool.tile([_K, _HALF], f16)
            v_sb = cpool.tile([_K, _N], f16)
            acc = cpool.tile([128, _N], f16)
            accT = cpool.tile([128, _N], f16)
            m1_sb = cpool.tile([128, _NT], f16)
            m2_sb = cpool.tile([128, _N // 128], f16)
            m64 = cpool.tile([128, _NT * 64], f16)

            # first matmul only needs u[:, :128] and v[:, :512]; land those
            # first, and stream the bulk on two DMA queues in parallel
            nc.sync.dma_start(u_sb[:, :128], u_d[:, :128])
            nc.sync.dma_start(v_sb[:, :512], v_d[:, :512])
            nc.sync.dma_start(v_sb[:, 512:2304], v_d[:, 512:2304])
            nc.gpsimd.dma_start(v_sb[:, 2304:], v_d[:, 2304:])
            nc.gpsimd.dma_start(u_sb[:, 128:], u_d[:, 128:])

            for t in range(_NT):
                # tile 0 converts straight into acc (saves memset + one max)
                D = acc if t == 0 else dpool.tile([128, _N], f16, name="D")
                for h in range(2):
                    ps = ppool.tile([128, 4 * _JC], f32, name="ps")
                    for c in range(4):
                        nc.tensor.matmul(
                            ps[:, ts(c, _JC)],
                            lhsT=u_sb[:, ts(t, 128)],
                            rhs=v_sb[:, ts(4 * h + c, _JC)],
                            start=True,
                            stop=True,
                        )
                    # convert fp32 PSUM -> negated fp16 SBUF
                    nc.scalar.activation(
                        D[:, ts(h, 4 * _JC)],
                        ps[:],
                        mybir.ActivationFunctionType.Copy,
                        scale=-1.0,
                    )
                # column direction first: the serial acc chain is the
                # critical dependency, keep it ahead of the fold work
                if t == _NT - 1:
                    # split the last update so the transpose tail can start
                    # on the first half while the second half finishes
                    nc.vector.tensor_tensor(
                        acc[:, : _N // 2], acc[:, : _N // 2], D[:, : _N // 2], MAX
                    )
                    nc.vector.tensor_tensor(
                        acc[:, _N // 2 :], acc[:, _N // 2 :], D[:, _N // 2 :], MAX
                    )
                elif t > 0:
                    nc.vector.tensor_tensor(acc[:], acc[:], D[:], MAX)
                # row maxes (= negated row mins of d2): log2 fold down to 64
                # wide; one grouped reduce finishes every 4 tiles
                F = fpool.tile([128, _N // 2], f16, name="F")
                G = gpool.tile([128, _N // 4], f16, name="G")
                if t == 0:
                    # fold each 2048-half separately so DVE work can begin
                    # right after the first convert instead of the second
                    for hh in range(2):
                        o = hh * 1024
                        nc.vector.tensor_tensor(
                            F[:, o : o + 1024],
                            D[:, ts(2 * hh, 1024)],
                            D[:, ts(2 * hh + 1, 1024)],
                            MAX,
                        )
                    w = _N // 2
                    src, dst = F, G
                else:
                    w = _N // 2
                    nc.vector.tensor_tensor(
                        F[:, :w], D[:, :w], D[:, w : 2 * w], MAX
                    )
                    src, dst = F, G
                while w > 128:
                    hw_ = w // 2
                    nc.vector.tensor_tensor(
                        dst[:, :hw_], src[:, :hw_], src[:, hw_:w], MAX
                    )
                    src, dst = dst, src
                    w = hw_
                nc.vector.tensor_tensor(
                    m64[:, ts(t, 64)], src[:, :64], src[:, 64:128], MAX
                )

            # one reduce finishes all 16 per-tile row maxes (runs inside the
            # tail's DMA-transpose shadow)
            nc.vector.tensor_reduce(
                m1_sb[:],
                m64[:].rearrange("p (g w) -> p g w", w=64),
                axis=mybir.AxisListType.X,
                op=MAX,
            )

            # collapse acc's partition axis
            if USE_DMAT_TAIL:
                # blocked DMA-xbar transposes of acc (quartered so each
                # transpose's latency overlaps the previous reduce and the
                # last acc update), then DVE free-axis maxes
                q = _N // 4
                nb = q // 128
                for qq in range(4):
                    nc.sync.dma_start_transpose(
                        accT[:, ts(qq, q)].rearrange("p (b c) -> p b c", c=128),
                        acc[:, ts(qq, q)],
                    )
                for qq in range(4):
                    nc.vector.tensor_reduce(
                        m2_sb[:, ts(qq, nb)],
                        accT[:, ts(qq, q)].rearrange("p (b c) -> p b c", c=128),
                        axis=mybir.AxisListType.X,
                        op=MAX,
                    )
            else:
                # PE transpose-mode matmul per block + per-block DVE max
                eye_sb = cpool.tile([128, 128], f16)
                nc.sync.dma_start(eye_sb[:], eye_d[:])
                for tb in range(_N // 128):
                    tps = ppool.tile([128, 128], f16, name="ps")
                    nc.tensor.transpose(tps[:], acc[:, ts(tb, 128)], eye_sb[:])
                    nc.vector.tensor_reduce(
                        m2_sb[:, tb : tb + 1],
                        tps[:],
                        axis=mybir.AxisListType.X,
                        op=MAX,
                    )
            nc.sync.dma_start(m1_d[:], m1_sb[:])
            nc.sync.dma_start(m2_d[:], m2_sb[:])
    nc.compile()
    return nc


def _get_program():
    global _prog_cache
    if _prog_cache is None:
        _prog_cache = _build_program()
    return _prog_cache


def _split16(x):
    hi = x.astype(np.float16)
    lo = (x - hi.astype(np.float32)).astype(np.float16)
    return hi, lo


def _make_uv(pts):
    """pts: [N, 3] fp32 -> (u [13, N] f16, v [13, N] f16) staging vectors."""
    n = pts.shape[0]
    s = np.sum(pts * pts, axis=-1, dtype=np.float32)
    sh, sl = _split16(s)
    ph, pl = _split16(pts)
    ones = np.ones((n,), np.float16)
    u = np.stack(
        [sh, sl, ones, ones,
         ph[:, 0], ph[:, 1], ph[:, 2],
         ph[:, 0], ph[:, 1], ph[:, 2],
         pl[:, 0], pl[:, 1], pl[:, 2]]
    )
    m2h = (-2.0 * ph.astype(np.float32)).astype(np.float16)
    m2l = (-2.0 * pl.astype(np.float32)).astype(np.float16)
    v = np.stack(
        [ones, ones, sh, sl,
         m2h[:, 0], m2h[:, 1], m2h[:, 2],
         m2l[:, 0], m2l[:, 1], m2l[:, 2],
         m2h[:, 0], m2h[:, 1], m2h[:, 2]]
    )
    return np.ascontiguousarray(u), np.ascontiguousarray(v)


def _combine(results):
    total = 0.0
    for b in range(_B):
        r0, r1 = results[2 * b], results[2 * b + 1]
        neg_min_a = np.concatenate(
            [
                r0["m1"].astype(np.float64).T.ravel(),
                r1["m1"].astype(np.float64).T.ravel(),
            ]
        )
        neg_min_b = np.maximum(
            r0["m2"].astype(np.float64).T.ravel(),
            r1["m2"].astype(np.float64).T.ravel(),
        )
        da = np.sqrt(np.clip(-neg_min_a, 0.0, None))
        db = np.sqrt(np.clip(-neg_min_b, 0.0, None))
        total += (da.sum() + db.sum()) / (2.0 * _N)
    return np.array(total / _B, dtype=np.float32)


def make_in_maps(pc1, pc2):
    pc1 = np.ascontiguousarray(np.asarray(pc1, dtype=np.float32))
    pc2 = np.ascontiguousarray(np.asarray(pc2, dtype=np.float32))
    in_maps = []
    for b in range(_B):
        u_full, _ = _make_uv(pc1[b])
        _, v_full = _make_uv(pc2[b])
        for hhalf in range(2):
            u = np.ascontiguousarray(u_full[:, hhalf * _HALF : (hhalf + 1) * _HALF])
            in_maps.append({"u": u, "v": v_full, "eye": np.eye(128, dtype=np.float16)})
    return in_maps


def kernel(pc1, pc2):
    global LAST_RESULT
    from concourse.bass_utils import run_bass_kernel_spmd

    nc = _get_program()
    in_maps = make_in_maps(pc1, pc2)
    res = run_bass_kernel_spmd(
        nc, in_maps, list(range(_NCORES)), trace=TRACE
    )
    LAST_RESULT = res
    return _combine(res.results)


# revision 59
# speedup vs baseline: 1.0042x; 1.0042x over previous
"""Symmetric Chamfer distance (Euclidean norm) on 8 Trainium2 NeuronCores.

Problem: pc1, pc2: [B=4, N=4096, D=3] fp32. For each batch, the reference
materializes the [N, N] distance matrix dist[i, j] = ||a_i - b_j||_2, takes
row-mins and col-mins, and averages. Output: fp32 scalar.

Strategy
--------
Sharding: core c handles (batch b = c//2, half h = c%2) -> 2048 a-points
(rows of the distance matrix) x all 4096 b-points.

Math: d2(i,j) = |a_i|^2 + |b_j|^2 - 2 a_i.b_j, computed on the TensorEngine
as a K=13 fp16 matmul using a hi/lo fp16 split of every operand
(x = hi + lo, both fp16, so hi*hi + hi*lo + lo*hi captures the fp32 product
to ~2^-24): d2 comes out fp32-exact in PSUM at full fp16 matmul speed
(1 cycle/row vs 4 for fp32).

The u (per a-point) / v (per b-point) staging vectors of the K=13 product
are a tiny O(B*N*D) layout+precision transform of the inputs, done on host
as part of sharding.

Per [128, 4096] row-block of d2 (one i-tile):
  - PE: 8 matmuls of [13,128]x[13,512] -> PSUM fp32 (4-bank groups)
  - ScalarE: 2x activation-Copy with scale=-1.0 -> SBUF fp16 = NEGATED d2
    (negation turns every min into a max; host flips signs at the end)
  - VectorE: log2-fold max (fp16, 2x mode) -> per-a-point row maxes, plus a
    running elementwise max into acc[128, 4096] for the per-b-point column
    direction
  - tail: one blocked DMA-xbar transpose of acc, then DVE free-axis maxes
    finish the per-b-point column mins on device

VectorE is the bottleneck engine (~90% busy): every d2 value crosses it
twice (fold + acc) at 2 fp16 elem/lane/cycle, and no other engine on trn2
can do elementwise/reduction min through this toolchain (gpsimd software
tensor ops are rejected by walrus codegen for the Pool engine).

Min over fp16(d2) followed by host-side sqrt is exact enough: sqrt is
monotone so min commutes, and fp16 rounding of d2 gives ~5e-4 relative
per-element noise that averages out over 4096 mins (measured end-to-end
relative error ~5e-7 vs the fp32 reference).

Host combine: per batch, min the two half-shard column vectors, flip signs,
clamp, sqrt, sum - O(N) work.
"""

import numpy as np

_B, _N, _D = 4, 4096, 3
_NCORES = 8
_HALF = _N // 2  # a-points per core
_K = 13          # contraction slots of the split-fp16 quadratic expansion
_NT = _HALF // 128  # 16 i-tiles per core
_JC = 512        # j-chunk per matmul (one PSUM bank of fp32)

TRACE = False            # test harness may flip before calling kernel()
LAST_RESULT = None       # BassKernelResults of the last run (for profiling)

USE_DMAT_TAIL = True     # DMA-xbar transpose tail (else: PE transpose tail)

_prog_cache = None


def _build_program():
    import concourse.bass as bass
    import concourse.mybir as mybir
    from concourse import bacc, tile

    f16 = mybir.dt.float16
    f32 = mybir.dt.float32
    ts = bass.ts
    MAX = mybir.AluOpType.max

    nc = bacc.Bacc(
        "TRN2",
        target_bir_lowering=False,
        debug=False,
        num_devices=_NCORES,
    )
    u_d = nc.declare_dram_parameter("u", [_K, _HALF], f16, isOutput=False)
    v_d = nc.declare_dram_parameter("v", [_K, _N], f16, isOutput=False)
    eye_d = nc.declare_dram_parameter("eye", [128, 128], f16, isOutput=False)
    m1_d = nc.declare_dram_parameter("m1", [128, _NT], f16, isOutput=True)
    m2_d = nc.declare_dram_parameter("m2", [128, _N // 128], f16, isOutput=True)

    with tile.TileContext(nc) as tc:
        with (
            tc.tile_pool(name="const", bufs=1) as cpool,
            tc.tile_pool(name="dpool", bufs=8) as dpool,
            tc.tile_pool(name="fpool", bufs=3) as fpool,
            tc.tile_pool(name="gpool", bufs=3) as gpool,
            tc.tile_pool(name="psum", bufs=2, space="PSUM") as ppool,
        ):
            u_sb = cpool.tile([_K, _HALF], f16)
            v_sb = cpool.tile([_K, _N], f16)
            acc = cpool.tile([128, _N], f16)
            accT = cpool.tile([128, _N], f16)
            m1_sb = cpool.tile([128, _NT], f16)
            m2_sb = cpool.tile([128, _N // 128], f16)
            m64 = cpool.tile([128, _NT * 64], f16)

            # first matmul only needs u[:, :128] and v[:, :512]; land those
            # first, and stream the bulk on two DMA queues in parallel
            nc.sync.dma_start(u_sb[:, :128], u_d[:, :128])
            nc.sync.dma_start(v_sb[:, :512], v_d[:, :512])
            nc.sync.dma_start(v_sb[:, 512:2304], v_d[:, 512:2304])
            nc.gpsimd.dma_start(v_sb[:, 2304:], v_d[:, 2304:])
            nc.gpsimd.dma_start(u_sb[:, 128:], u_d[:, 128:])

            for t in range(_NT):
                # tile 0 converts straight into acc (saves memset + one max)
                D = acc if t == 0 else dpool.tile([128, _N], f16, name="D")
                for h in range(2):
                    ps = ppool.tile([128, 4 * _JC], f32, name="ps")
                    for c in range(4):
                        nc.tensor.matmul(
                            ps[:, ts(c, _JC)],
                            lhsT=u_sb[:, ts(t, 128)],
                            rhs=v_sb[:, ts(4 * h + c, _JC)],
                            start=True,
                            stop=True,
                        )
                    # convert fp32 PSUM -> negated fp16 SBUF
                    nc.scalar.activation(
                        D[:, ts(h, 4 * _JC)],
                        ps[:],
                        mybir.ActivationFunctionType.Copy,
                        scale=-1.0,
                    )
                # column direction first: the serial acc chain is the
                # critical dependency, keep it ahead of the fold work
                if t == _NT - 1:
                    # split the last update so the transpose tail can start
                    # on the first half while the second half finishes
                    nc.vector.tensor_tensor(
                        acc[:, : _N // 2], acc[:, : _N // 2], D[:, : _N // 2], MAX
                    )
                    nc.vector.tensor_tensor(
                        acc[:, _N // 2 :], acc[:, _N // 2 :], D[:, _N // 2 :], MAX
                    )
                elif t > 0:
                    nc.vector.tensor_tensor(acc[:], acc[:], D[:], MAX)
                # row maxes (= negated row mins of d2): log2 fold down to 64
                # wide; one grouped reduce finishes every 4 tiles
                F = fpool.tile([128, _N // 2], f16, name="F")
                G = gpool.tile([128, _N // 4], f16, name="G")
                if t == 0:
                    # fold each 2048-half separately so DVE work can begin
                    # right after the first convert instead of the second
                    for hh in range(2):
                        o = hh * 1024
                        nc.vector.tensor_tensor(
                            F[:, o : o + 1024],
                            D[:, ts(2 * hh, 1024)],
                            D[:, ts(2 * hh + 1, 1024)],
                            MAX,
                        )
                    w = _N // 2
                    src, dst = F, G
                else:
                    w = _N // 2
                    nc.vector.tensor_tensor(
                        F[:, :w], D[:, :w], D[:, w : 2 * w], MAX
                    )
                    src, dst = F, G
                while w > 128:
                    hw_ = w // 2
                    nc.vector.tensor_tensor(
                        dst[:, :hw_], src[:, :hw_], src[:, hw_:w], MAX
                    )
                    src, dst = dst, src
                    w = hw_
                nc.vector.tensor_tensor(
                    m64[:, ts(t, 64)], src[:, :64], src[:, 64:128], MAX
                )

            # one reduce finishes all 16 per-tile row maxes (runs inside the
            # tail's DMA-transpose shadow)
            nc.vector.tensor_reduce(
                m1_sb[:],
                m64[:].rearrange("p (g w) -> p g w", w=64),
                axis=mybir.AxisListType.X,
                op=MAX,
            )

            # collapse acc's partition axis
            if USE_DMAT_TAIL:
                # blocked DMA-xbar transposes of acc (quartered so each
                # transpose's latency overlaps the previous reduce and the
                # last acc update), then DVE free-axis maxes
                q = _N // 4
                nb = q // 128
                for qq in range(4):
                    nc.sync.dma_start_transpose(
                        accT[:, ts(qq, q)].rearrange("p (b c) -> p b c", c=128),
                        acc[:, ts(qq, q)],
                    )
                for qq in range(4):
                    nc.vector.tensor_reduce(
                        m2_sb[:, ts(qq, nb)],
                        accT[:, ts(qq, q)].rearrange("p (b c) -> p b c", c=128),
                        axis=mybir.AxisListType.X,
                        op=MAX,
                    )
            else:
                # PE transpose-mode matmul per block + per-block DVE max
                eye_sb = cpool.tile([128, 128], f16)
                nc.sync.dma_start(eye_sb[:], eye_d[:])
                for tb in range(_N // 128):
                    tps = ppool.tile([128, 128], f16, name="ps")
                    nc.tensor.transpose(tps[:], acc[:, ts(tb, 128)], eye_sb[:])
                    nc.vector.tensor_reduce(
                        m2_sb[:, tb : tb + 1],
                        tps[:],
                        axis=mybir.AxisListType.X,
                        op=MAX,
                    )
            nc.sync.dma_start(m1_d[:], m1_sb[:])
            nc.sync.dma_start(m2_d[:], m2_sb[:])
    nc.compile()
    return nc


def _get_program():
    global _prog_cache
    if _prog_cache is None:
        _prog_cache = _build_program()
    return _prog_cache


def _split16(x):
    hi = x.astype(np.float16)
    lo = (x - hi.astype(np.float32)).astype(np.float16)
    return hi, lo


def _make_uv(pts):
    """pts: [N, 3] fp32 -> (u [13, N] f16, v [13, N] f16) staging vectors."""
    n = pts.shape[0]
    s = np.sum(pts * pts, axis=-1, dtype=np.float32)
    sh, sl = _split16(s)
    ph, pl = _split16(pts)
    ones = np.ones((n,), np.float16)
    u = np.stack(
        [sh, sl, ones, ones,
         ph[:, 0], ph[:, 1], ph[:, 2],
         ph[:, 0], ph[:, 1], ph[:, 2],
         pl[:, 0], pl[:, 1], pl[:, 2]]
    )
    m2h = (-2.0 * ph.astype(np.float32)).astype(np.float16)
    m2l = (-2.0 * pl.astype(np.float32)).astype(np.float16)
    v = np.stack(
        [ones, ones, sh, sl,
         m2h[:, 0], m2h[:, 1], m2h[:, 2],
         m2l[:, 0], m2l[:, 1], m2l[:, 2],
         m2h[:, 0], m2h[:, 1], m2h[:, 2]]
    )
    return np.ascontiguousarray(u), np.ascontiguousarray(v)


def _combine(results):
    total = 0.0
    for b in range(_B):
        r0, r1 = results[2 * b], results[2 * b + 1]
        neg_min_a = np.concatenate(
            [
                r0["m1"].astype(np.float64).T.ravel(),
                r1["m1"].astype(np.float64).T.ravel(),
            ]
        )
        neg_min_b = np.maximum(
            r0["m2"].astype(np.float64).T.ravel(),
            r1["m2"].astype(np.float64).T.ravel(),
        )
        da = np.sqrt(np.clip(-neg_min_a, 0.0, None))
        db = np.sqrt(np.clip(-neg_min_b, 0.0, None))
        total += (da.sum() + db.sum()) / (2.0 * _N)
    return np.array(total / _B, dtype=np.float32)


def make_in_maps(pc1, pc2):
    pc1 = np.ascontiguousarray(np.asarray(pc1, dtype=np.float32))
    pc2 = np.ascontiguousarray(np.asarray(pc2, dtype=np.float32))
    in_maps = []
    for b in range(_B):
        u_full, _ = _make_uv(pc1[b])
        _, v_full = _make_uv(pc2[b])
        for hhalf in range(2):
            u = np.ascontiguousarray(u_full[:, hhalf * _HALF : (hhalf + 1) * _HALF])
            in_maps.append({"u": u, "v": v_full, "eye": np.eye(128, dtype=np.float16)})
    return in_maps


def kernel(pc1, pc2):
    global LAST_RESULT
    from concourse.bass_utils import run_bass_kernel_spmd

    nc = _get_program()
    in_maps = make_in_maps(pc1, pc2)
    res = run_bass_kernel_spmd(
        nc, in_maps, list(range(_NCORES)), trace=TRACE
    )
    LAST_RESULT = res
    return _combine(res.results)
